# revision 73
# baseline (speedup 1.0000x reference)
"""AttentionReadout kernel for Trainium2 (8 NeuronCores, data-parallel by chunk).

Reference computation (per full input):
    scores = (tanh(x @ W1 + b1) @ W2)[:, 0]          # [N]
    chunk_id = batch // 32                            # 32 graphs per chunk
    w = softmax of scores within each chunk           # [N]
    out = segment_sum(w[:, None] * x, batch)          # [4096, 256]

Shapes: x [262144, 256] f32, batch [262144] i64 (sorted, uniform: 64
nodes/graph), W1 [256,256], b1 [256], W2 [256,1].

Strategy (per core, 32768 nodes = 16 chunks of 2048 nodes):
  - host: ship x twice -- natural layout in bf16 (pooling; needs the
    precision) and transposed layout in fp8e4m3 (MLP scores; tolerates the
    quantization: the softmax weights absorb it, measured rel err ~1e-2
    against the 2e-2 gate).  24 MB/core instead of 32 MB bf16x2.
  - device, per chunk:
      hT = W1.T @ x8T         (PE, fp8 DoubleRow or bf16xfp8, psum f32)
      th = tanh(hT + b1)      (ACT, psum->sbuf bf16)
      s[n] = th.T @ W2        (PE, tanh tile as stationary operand -> s in
                               node-partition layout, F=1, nearly free)
      e = exp(s)              (ACT, one instr per chunk QUAD -- pairs and
                               singles for the tail chunks; DVE sums rows)
      D = allreduce(rowsum)   (GPSIMD partition_all_reduce, off critical path)
      E[n, g] = e * mask      (DVE, UNNORMALIZED; mask precomputed on host)
      outT[h,g] += x_t.T @ E  (PE, x_nat tile stationary, E moving F=32;
                               output transposed [256, 32] -- host undoes it)
      out = outT * (1/D)      (DVE, folded into the psum->sbuf output copy:
                               1/D is identical on every partition)
  - softmax max-subtraction is skipped: scores = tanh(.)@W2 are bounded by
    sum|W2| <= 16, so exp() cannot overflow in f32 and w = e/sum(e) is
    mathematically identical to the max-shifted form.
  - scheduling: the kernel is jointly DMA- and ACT-bound (both ~70us/core).
    The fp8 xt stream runs XT_LEAD chunks ahead of the bf16 x_nat stream on
    the serial DMA device so the tanh pipeline never starves; the last
    TAIL_SPLIT_CHUNKS x_nat slabs are quartered so the final poolings chase
    the arriving tiles, shortening the post-load serial tail; exp is
    quad-batched with pool-psum allocations staggered over two iterations
    to fit the 8-bank PSUM budget.
"""

import numpy as np
import ml_dtypes

import concourse.bass as bass
import concourse.bacc as bacc
import concourse.tile as tile
import concourse.mybir as mybir
import concourse.bass_isa as bass_isa
from concourse.bass_utils import run_bass_kernel_spmd

BF16 = mybir.dt.bfloat16
FP8 = mybir.dt.float8e4
F32 = mybir.dt.float32
NP_BF16 = ml_dtypes.bfloat16
NP_FP8 = ml_dtypes.float8_e4m3fn

N_CORES = 8
HIDDEN = 256
CHUNK_GRAPHS = 32
GRAPH_NODES = 64          # uniform: nodes per graph
TILE_NODES = 128          # nodes per node-tile (SBUF partition dim)
CHUNK_NODES = CHUNK_GRAPHS * GRAPH_NODES      # 2048
TILES_PER_CHUNK = CHUNK_NODES // TILE_NODES   # 16
BLOCK_NODES = 512
XT_LEAD = 6
TAIL_SPLIT_CHUNKS = 6
TAIL_SPLIT_WAYS = 4

# MLP matmul flavor: "mixed" = bf16 W1 stationary x fp8 moving;
# "dr" = fp8 DoubleRow (both operands fp8, K=256 in one matmul, 2x rate)
MLP_MODE = "dr"

_NC_CACHE = {}


def build_nc(n_chunks, mlp_mode=MLP_MODE):
    """Build the per-core Bass program (identical across cores)."""
    assert n_chunks % 2 == 0
    nc = bacc.Bacc("TRN2", target_bir_lowering=False, debug=False,
                   enable_asserts=False)

    nodes = n_chunks * CHUNK_NODES
    # DRAM I/O (per-core shard)
    x_nat_d = nc.dram_tensor(
        "x_nat", [n_chunks, TILE_NODES, TILES_PER_CHUNK, HIDDEN], BF16,
        kind="ExternalInput").ap()
    # transposed fp8 layout: [q, kt, node] with k = kt*128 + q
    x8t_d = nc.dram_tensor(
        "x8t", [128, 2, nodes], FP8, kind="ExternalInput").ap()
    w1_dt = FP8 if mlp_mode == "dr" else BF16
    w1_d = nc.dram_tensor("w1", [128, 2, 2, 128], w1_dt,
                          kind="ExternalInput").ap()
    w2_d = nc.dram_tensor("w2", [128, 2], BF16, kind="ExternalInput").ap()
    b1_d = nc.dram_tensor("b1", [128, 2], F32, kind="ExternalInput").ap()
    mask_d = nc.dram_tensor(
        "maskw", [TILE_NODES, CHUNK_GRAPHS, TILES_PER_CHUNK], BF16,
        kind="ExternalInput").ap()
    # transposed output, two chunks per row-group: [cpair, p=h, ci, kt, g]
    out_d = nc.dram_tensor(
        "outt", [n_chunks // 2, 128, 2, 2, CHUNK_GRAPHS], F32,
        kind="ExternalOutput").ap()

    with tile.TileContext(nc) as tc:
        with (
            tc.tile_pool(name="consts", bufs=1) as consts,
            tc.tile_pool(name="xpool", bufs=7) as xpool,
            tc.tile_pool(name="xtpool", bufs=9) as xtpool,
            tc.tile_pool(name="thpool", bufs=10) as thpool,
            tc.tile_pool(name="epool", bufs=5) as epool,
            tc.tile_pool(name="opool", bufs=8) as opool,
            tc.tile_pool(name="hpsum", bufs=2, space="PSUM") as hpsum,
            tc.tile_pool(name="spsum", bufs=2, space="PSUM") as spsum,
            tc.tile_pool(name="ppsum", bufs=2, space="PSUM") as ppsum,
        ):
            st = {}        # per-chunk live tiles
            spair = {}     # per-pair score psum tiles

            HALF = CHUNK_NODES // 2

            def load_xt(c):
                # one full-slab DMA per chunk: a 1.46us transfer outruns the
                # ~1.2us single-queue dispatch cadence, so the serial DMA
                # device never starves during the xt-lead fill (half-slab
                # transfers did starve it).  The MLP still consumes by
                # half-slices of the one tile.
                t = xtpool.tile([128, 2, CHUNK_NODES], FP8, tag="xt",
                                name=f"xt{c}")
                lo = c * CHUNK_NODES
                nc.sync.dma_start(out=t, in_=x8t_d[:, :, lo:lo + CHUNK_NODES])
                st[c] = {"xt": (t[:, :, 0:HALF], t[:, :, HALF:]), "th": {}}

            def load_xnat(c):
                x_sb = xpool.tile([TILE_NODES, TILES_PER_CHUNK, HIDDEN], BF16,
                                  tag="x")
                if c >= n_chunks - TAIL_SPLIT_CHUNKS:
                    # tail chunks: sliced loads so their pooling matmuls
                    # chase the arriving tiles instead of waiting the slab
                    q = TILES_PER_CHUNK // TAIL_SPLIT_WAYS
                    for qi in range(TAIL_SPLIT_WAYS):
                        nc.sync.dma_start(
                            out=x_sb[:, qi * q:(qi + 1) * q],
                            in_=x_nat_d[c, :, qi * q:(qi + 1) * q])
                else:
                    nc.sync.dma_start(out=x_sb, in_=x_nat_d[c])
                st[c]["x"] = x_sb

            # first MLP inputs go ahead of everything else on the SP queue;
            # consts dispatch concurrently through the Activation HWDGE queue
            # chunk 0: w1 rides the serial DMA stream between the two xt
            # halves, so the first MLP matmul (xt half 0 + w1) starts ~1.3us
            # earlier than if w1 queued behind both halves
            xt0 = xtpool.tile([128, 2, CHUNK_NODES], FP8, tag="xt",
                              name="xt0")
            nc.sync.dma_start(out=xt0, in_=x8t_d[:, :, 0:CHUNK_NODES])
            w1_sb = consts.tile([128, 2, 2, 128], w1_dt)
            nc.sync.dma_start(out=w1_sb, in_=w1_d)
            st[0] = {"xt": (xt0[:, :, 0:HALF], xt0[:, :, HALF:]), "th": {}}
            b1_sb = consts.tile([128, 2], F32)
            nc.scalar.dma_start(out=b1_sb, in_=b1_d)
            # activation-table preload: dummy tanh+exp on an already-loaded
            # const so the (shared) table load hides under the DMA fill
            scratch = consts.tile([128, 1], BF16, name="scratch")
            nc.scalar.activation(scratch, b1_sb[:, 0:1],
                                 mybir.ActivationFunctionType.Tanh)
            nc.scalar.activation(scratch, b1_sb[:, 0:1],
                                 mybir.ActivationFunctionType.Exp)
            # w2 (first used by scores in iteration 1) and the mask (first
            # used by the iteration-2 softmax) are dispatched from inside
            # iteration 1: their tiny transfers would otherwise steal HWDGE
            # dispatch slots from the xt stream in the startup window
            w2_sb = consts.tile([128, 2], BF16, name="w2_sb")
            mask_sb = consts.tile([TILE_NODES, CHUNK_GRAPHS, TILES_PER_CHUNK],
                                  BF16)
            # xt runs XT_LEAD chunks ahead of xnat on the wire: the tanh
            # stream (the critical engine) consumes xt at almost exactly the
            # DMA rate, so the lead absorbs the accumulated per-chunk DMA
            # deficit (stores, dispatch bubbles) that would otherwise stall
            # ACT in the back half.  xnat still flows every iteration so the
            # pooling matmuls never clog the PE queue waiting for it.
            if n_chunks > 1:
                load_xt(1)
            load_xnat(0)
            for cc in range(2, min(XT_LEAD + 1, n_chunks)):
                load_xt(cc)
            if n_chunks > 1:
                load_xnat(1)

            # Software pipeline (pair-structured):
            #   iteration c emits, round-robin per node-tile slot:
            #     MLP matmuls of chunk c        (+ tanh on ACT)
            #     score matmuls of c-1          (F=1, nearly free)
            #     pooling matmuls of pair (c-3, c-2) on odd c
            #   after the slots: softmax of pair (c-2, c-1) on even c,
            #   output store of pair (c-3, c-2) on odd c.

            # chunks below QEND batch their softmax per QUAD (one exp for
            # 4 chunks): fewer ACT instructions on the critical tanh stream
            QEND = ((n_chunks - 4) // 4) * 4 if n_chunks >= 8 else 0

            def mlp_gen(c):
                """Yields after each MLP matmul slot (16 total)."""
                if c < QEND:
                    if c % 4 == 0:
                        spair[("q", c // 4)] = spsum.tile(
                            [128, 4, TILES_PER_CHUNK], F32, tag="s",
                            name=f"s_q{c // 4}")
                elif c % 2 == 0:
                    spair[c // 2] = spsum.tile([128, 2, TILES_PER_CHUNK], F32,
                                               tag="s", name=f"s_ps{c // 2}")
                for bp in range(2):          # block pair: nodes [bp*1024, ...)
                    xt_sb = st[c]["xt"][bp]
                    for mt in range(2):
                        h_ps = hpsum.tile([128, 2, BLOCK_NODES], F32, tag="h",
                                          name=f"h_ps{c}_{bp}_{mt}")
                        for bb in range(2):
                            nlo = bb * BLOCK_NODES
                            if mlp_mode == "dr":
                                nc.tensor.matmul(
                                    h_ps[:, bb, :], w1_sb[:, :, mt, :],
                                    xt_sb[:, :, nlo:nlo + BLOCK_NODES],
                                    perf_mode=mybir.MatmulPerfMode.DoubleRow,
                                    start=True, stop=True)
                                yield
                                yield
                            else:
                                for kt in range(2):
                                    nc.tensor.matmul(
                                        h_ps[:, bb, :], w1_sb[:, kt, mt, :],
                                        xt_sb[:, kt, nlo:nlo + BLOCK_NODES],
                                        start=(kt == 0), stop=(kt == 1))
                                    yield
                        th = thpool.tile([128, 2, BLOCK_NODES], BF16, tag="th",
                                         name=f"th{c}_{bp}_{mt}")
                        nc.scalar.activation(
                            th, h_ps, mybir.ActivationFunctionType.Tanh,
                            bias=b1_sb[:, mt:mt + 1], scale=1.0)
                        st[c]["th"][(bp, mt)] = th

            def score_ops(c):
                """16 slots; each slot t emits 2 accumulating F=1 matmuls
                (tanh tile as stationary operand)."""
                if c < QEND:
                    s_ps = spair[("q", c // 4)][:, c % 4]
                else:
                    s_ps = spair[c // 2][:, c % 2]
                ops = []
                for t in range(TILES_PER_CHUNK):
                    b, tl = divmod(t, 4)
                    bp, bb = divmod(b, 2)

                    def op(mt, t=t, bp=bp, bb=bb, c=c, tl=tl):
                        th = st[c]["th"][(bp, mt)]
                        nc.tensor.matmul(
                            s_ps[:, t:t + 1],
                            th[:, bb, tl * 128:(tl + 1) * 128],
                            w2_sb[:, mt:mt + 1],
                            start=(mt == 0), stop=(mt == 1))
                    ops.append((lambda op=op: op(0), lambda op=op: op(1)))
                return ops

            def _softmax_tail(c, e_view, rden, ci, alloc_p=True):
                # E' = e * mask, UNNORMALIZED: 1/D is folded into the
                # psum->sbuf output copy instead (rden holds the same value
                # on every partition), so pooling never waits the
                # allreduce/reciprocal chain
                e_full = epool.tile(
                    [TILE_NODES, CHUNK_GRAPHS, TILES_PER_CHUNK], BF16,
                    tag="efull", name=f"efull{c}")
                e_bc = e_view.unsqueeze(1).broadcast_to(
                    [TILE_NODES, CHUNK_GRAPHS, TILES_PER_CHUNK])
                nc.vector.tensor_mul(e_full, e_bc, mask_sb)
                st[c]["E"] = e_full
                st[c]["rden"] = (rden, ci)
                if alloc_p:
                    emit_palloc(c)

            def emit_palloc(c):
                p_ps = ppsum.tile([128, 2, CHUNK_GRAPHS], F32, tag="p",
                                  name=f"p_ps{c}")
                st[c]["p"] = p_ps

            def emit_softmax_quad(qi):
                """exp + normalization for chunks 4qi..4qi+3 in one batch.
                Only the first two chunks get their pool-psum tiles now; the
                other two are allocated next iteration (8-bank budget)."""
                e_sb = epool.tile([128, 4, TILES_PER_CHUNK], BF16, tag="e",
                                  name=f"e_q{qi}")
                nc.scalar.activation(
                    e_sb, spair[("q", qi)], mybir.ActivationFunctionType.Exp)
                del spair[("q", qi)]
                acc = epool.tile([128, 4], F32, tag="acc", name=f"acc_q{qi}")
                nc.vector.tensor_reduce(
                    acc, e_sb, mybir.AxisListType.X, mybir.AluOpType.add)
                dsum = epool.tile([128, 4], F32, tag="dsum",
                                  name=f"dsum_q{qi}")
                nc.gpsimd.partition_all_reduce(
                    dsum, acc, 128, bass_isa.ReduceOp.add)
                rden = epool.tile([128, 4], F32, tag="rden",
                                  name=f"rden_q{qi}")
                nc.vector.reciprocal(rden, dsum)
                for ci in range(4):
                    _softmax_tail(4 * qi + ci, e_sb[:, ci], rden, ci,
                                  alloc_p=(ci < 2))

            def emit_softmax_pair(p):
                """exp + normalization for chunks (2p, 2p+1) in one batch."""
                e_sb = epool.tile([128, 2, TILES_PER_CHUNK], BF16, tag="e",
                                  name=f"e_sb{p}")
                nc.scalar.activation(
                    e_sb, spair[p], mybir.ActivationFunctionType.Exp)
                del spair[p]
                acc = epool.tile([128, 2], F32, tag="acc")
                nc.vector.tensor_reduce(
                    acc, e_sb, mybir.AxisListType.X, mybir.AluOpType.add)
                dsum = epool.tile([128, 2], F32, tag="dsum")
                nc.gpsimd.partition_all_reduce(
                    dsum, acc, 128, bass_isa.ReduceOp.add)
                rden = epool.tile([128, 2], F32, tag="rden")
                nc.vector.reciprocal(rden, dsum)
                for ci in range(2):
                    _softmax_tail(2 * p + ci, e_sb[:, ci], rden, ci)

            def emit_softmax_single(c):
                """Per-chunk softmax for the tail chunks: avoids coupling the
                last chunks' pooling to the final pair exp."""
                p, ci = divmod(c, 2)
                e_sb = epool.tile([128, TILES_PER_CHUNK], BF16, tag="e",
                                  name=f"e_sb_s{c}")
                nc.scalar.activation(
                    e_sb, spair[p][:, ci], mybir.ActivationFunctionType.Exp)
                if ci == 1:
                    del spair[p]
                acc = epool.tile([128, 1], F32, tag="acc", name=f"acc_s{c}")
                nc.vector.tensor_reduce(
                    acc, e_sb, mybir.AxisListType.X, mybir.AluOpType.add)
                dsum = epool.tile([128, 1], F32, tag="dsum", name=f"dsum_s{c}")
                nc.gpsimd.partition_all_reduce(
                    dsum, acc, 128, bass_isa.ReduceOp.add)
                rden = epool.tile([128, 1], F32, tag="rden", name=f"rden_s{c}")
                nc.vector.reciprocal(rden, dsum)
                _softmax_tail(c, e_sb, rden, 0)

            def pool_chunk_ops(cs):
                """16 slots; slot t emits 2 matmuls per chunk in cs (kt
                halves): stationary = x_nat tile [128 nodes, 128 h], moving =
                E[:, :, t] (F=32), accumulated over the 16 node-tiles into
                outT [128, 2, 32] per chunk.  Both kt slices share one 2KB
                psum zero region, so exactly one start and one stop per
                chunk tile: a second start would re-mark the region
                pending-zero and wipe the first group's partial sums."""
                ops = []
                for t in range(TILES_PER_CHUNK):
                    def op(t=t):
                        for c in cs:
                            for kt in range(2):
                                nc.tensor.matmul(
                                    st[c]["p"][:, kt, :],
                                    st[c]["x"][:, t, kt * 128:(kt + 1) * 128],
                                    st[c]["E"][:, :, t],
                                    start=(t == 0 and kt == 0),
                                    stop=(t == TILES_PER_CHUNK - 1 and
                                          kt == 1),
                                    skip_group_check=True)
                    ops.append(op)
                return ops

            deferred = []

            def emit_store_pair(p):
                o2 = opool.tile([128, 2, 2, CHUNK_GRAPHS], F32,
                                tag="o", name=f"o2_{p}")
                for ci in range(2):
                    rden, ri = st[2 * p + ci]["rden"]
                    nc.vector.tensor_scalar_mul(
                        o2[:, ci], st[2 * p + ci]["p"], rden[:, ri:ri + 1])
                # defer ALL pair-store dispatches until the last pair's
                # copies are emitted: early stores would steal serial
                # DMA-engine slots from the load stream and delay the last
                # load byte, which gates the terminal pool chain.  By then
                # the SP queue has no loads left to block.
                deferred.append((p, o2))
                if p == lastp - 1:
                    for dp, do2 in deferred:
                        nc.sync.dma_start(out=out_d[dp], in_=do2)
                    deferred.clear()
                del st[2 * p]
                del st[2 * p + 1]

            lastp = n_chunks // 2 - 1
            for c in range(n_chunks + 2):
                if c == min(1, n_chunks - 1):
                    nc.scalar.dma_start(out=w2_sb, in_=w2_d)
                    nc.scalar.dma_start(out=mask_sb, in_=mask_d)
                if c + XT_LEAD + 1 < n_chunks:
                    load_xt(c + XT_LEAD + 1)
                if c + 2 < n_chunks:
                    load_xnat(c + 2)
                # Sequential emission per iteration: PE executes in order, so
                # score ops (which wait on tanh of c-1) must not sit ahead of
                # MLP matmuls whose inputs are already resident.
                if c < n_chunks:
                    for _ in mlp_gen(c):
                        pass
                if 1 <= c <= n_chunks:
                    for op0, op1 in score_ops(c - 1):
                        op0()
                        op1()
                if c % 4 == 1 and 5 <= c <= QEND + 1:
                    # quad first half: chunks (c-5, c-4)
                    for op in pool_chunk_ops([c - 5, c - 4]):
                        op()
                    emit_store_pair((c - 5) // 2)
                    emit_palloc(c - 3)
                    emit_palloc(c - 2)
                elif c % 4 == 2 and 6 <= c <= QEND + 2:
                    # quad second half: chunks (c-4, c-3)
                    for op in pool_chunk_ops([c - 4, c - 3]):
                        op()
                    emit_store_pair((c - 4) // 2)
                elif c >= 3 and c % 2 == 1 and QEND // 2 <= (c - 3) // 2 < lastp:
                    for op in pool_chunk_ops([c - 3, c - 2]):
                        op()
                    emit_store_pair((c - 3) // 2)
                elif c == n_chunks:
                    for op in pool_chunk_ops([n_chunks - 2]):
                        op()
                elif c == n_chunks + 1:
                    for op in pool_chunk_ops([n_chunks - 1]):
                        op()
                if c % 4 == 0 and 4 <= c <= QEND:
                    emit_softmax_quad((c - 4) // 4)
                if c >= 2 and c % 2 == 0 and QEND // 2 <= (c - 2) // 2 < lastp:
                    emit_softmax_pair((c - 2) // 2)
                if c == n_chunks - 1:
                    emit_softmax_single(n_chunks - 2)
                if c == n_chunks:
                    # copy chunk n-2's pooled output while n-1 still cooks
                    o2l = opool.tile([128, 2, 2, CHUNK_GRAPHS], F32,
                                     tag="o", name="o2_last")
                    st["o2l"] = o2l
                    rden, ri = st[n_chunks - 2]["rden"]
                    nc.vector.tensor_scalar_mul(
                        o2l[:, 0], st[n_chunks - 2]["p"], rden[:, ri:ri + 1])
                    emit_softmax_single(n_chunks - 1)
                if c == n_chunks + 1:
                    o2l = st["o2l"]
                    rden, ri = st[n_chunks - 1]["rden"]
                    nc.vector.tensor_scalar_mul(
                        o2l[:, 1], st[n_chunks - 1]["p"], rden[:, ri:ri + 1])
                    # final store via SP HWDGE: lower latency than SWDGE and
                    # nothing is left in the SP queue to block
                    nc.sync.dma_start(out=out_d[lastp], in_=o2l)

    nc.compile()
    return nc


def _prep_inputs(x, W1, b1, W2, n_chunks_per_core, mlp_mode=MLP_MODE):
    """Host-side marshalling: casts, layouts, masks. Returns in_maps."""
    N, H = x.shape
    nodes_per_core = n_chunks_per_core * CHUNK_NODES

    xf = np.asarray(x, dtype=np.float32)
    xb = xf.astype(NP_BF16)

    # natural layout: [core, chunk, p, t, h]  (bf16: pooling precision)
    x_nat = np.ascontiguousarray(
        xb.reshape(N_CORES, n_chunks_per_core, TILES_PER_CHUNK, TILE_NODES, H)
        .transpose(0, 1, 3, 2, 4))
    # transposed fp8 layout: [core, q, kt, n_local] with k = kt*128 + q
    x8t = np.ascontiguousarray(
        xf.reshape(N_CORES, nodes_per_core, 2, 128)
        .transpose(0, 3, 2, 1)).astype(NP_FP8)

    np_w1 = NP_FP8 if mlp_mode == "dr" else NP_BF16
    W1c = np.asarray(W1, dtype=np.float32).astype(np_w1)     # [hin, hout]
    w1_host = np.ascontiguousarray(
        W1c.reshape(2, 128, 2, 128).transpose(1, 0, 2, 3))  # [q, kt, mt, j]
    w2_host = np.ascontiguousarray(
        np.asarray(W2).astype(NP_BF16).reshape(2, 128).T)   # [p, mt]
    b1_host = np.ascontiguousarray(
        np.asarray(b1).astype(np.float32).reshape(2, 128).T)  # [p, mt]

    # mask[p, g, t] = 1 iff node (t, p) of a chunk belongs to graph g
    p_idx = np.arange(TILE_NODES)
    t_idx = np.arange(TILES_PER_CHUNK)
    g_of_pt = 2 * t_idx[None, :] + p_idx[:, None] // GRAPH_NODES  # [p, t]
    mask_host = (g_of_pt[:, None, :] ==
                 np.arange(CHUNK_GRAPHS)[None, :, None]).astype(NP_BF16)

    in_maps = []
    for core in range(N_CORES):
        in_maps.append({
            "x_nat": x_nat[core],
            "x8t": x8t[core],
            "w1": w1_host,
            "w2": w2_host,
            "b1": b1_host,
            "maskw": mask_host,
        })
    return in_maps


def _reference_numpy(x, batch, W1, b1, W2):
    """Fallback for non-uniform batch layouts: straight numpy."""
    x = np.asarray(x, dtype=np.float64)
    batch = np.asarray(batch).astype(np.int64)
    # the reference uses a fixed segment count (num_graphs = num_nodes/64),
    # not batch.max()+1 -- keep trailing empty graphs as zero rows
    n_graphs = max(int(batch.max()) + 1, x.shape[0] // GRAPH_NODES)
    scores = np.tanh(x @ np.asarray(W1, np.float64) +
                     np.asarray(b1, np.float64)) @ np.asarray(W2, np.float64)
    scores = scores[:, 0]
    chunk_id = batch // CHUNK_GRAPHS
    n_chunks = int(chunk_id.max()) + 1
    m = np.full(n_chunks, -np.inf)
    np.maximum.at(m, chunk_id, scores)
    e = np.exp(scores - m[chunk_id])
    denom = np.zeros(n_chunks)
    np.add.at(denom, chunk_id, e)
    w = e / denom[chunk_id]
    out = np.zeros((n_graphs, x.shape[1]))
    np.add.at(out, batch, w[:, None] * x)
    return out.astype(np.float32)


def kernel(x, batch, W1, b1, W2, trace=False):
    x = np.asarray(x)
    batch = np.asarray(batch)
    N, H = x.shape
    n_graphs = int(batch[-1]) + 1

    # This kernel is specialized for the uniform sorted batch that the
    # reference generator produces (64 nodes per graph). Anything else
    # falls back to a host computation.
    expected = (np.arange(N, dtype=np.int64) * n_graphs) // N
    if (H != HIDDEN or N % (N_CORES * CHUNK_NODES) != 0
            or n_graphs % (N_CORES * CHUNK_GRAPHS) != 0
            or (N // (N_CORES * CHUNK_NODES)) % 2 != 0
            or not np.array_equal(batch.astype(np.int64), expected)):
        return _reference_numpy(x, batch, W1, b1, W2)

    n_chunks_per_core = N // (N_CORES * CHUNK_NODES)

    key = (n_chunks_per_core, MLP_MODE)
    if key not in _NC_CACHE:
        _NC_CACHE[key] = build_nc(n_chunks_per_core, mlp_mode=MLP_MODE)
    nc = _NC_CACHE[key]

    in_maps = _prep_inputs(x, W1, b1, W2, n_chunks_per_core,
                           mlp_mode=MLP_MODE)
    try:
        res = run_bass_kernel_spmd(nc, in_maps, core_ids=list(range(N_CORES)),
                                   trace=trace)
    except ModuleNotFoundError:
        # NTFF trace hooks unavailable in this environment
        res = run_bass_kernel_spmd(nc, in_maps, core_ids=list(range(N_CORES)),
                                   trace=False)
    # outt [cpair, p, ci, kt, g] -> out[g, h]: g=(2cp+ci)*32+gg, h=kt*128+p
    outs = []
    for r in res.results:
        arr = r["outt"]
        outs.append(np.ascontiguousarray(
            arr.transpose(0, 2, 4, 3, 1)).reshape(-1, HIDDEN))
    out = np.concatenate(outs, axis=0)
    if trace:
        kernel.last_results = res
    return out.astype(np.float32)



# revision 77
# speedup vs baseline: 1.0013x; 1.0013x over previous
"""AttentionReadout kernel for Trainium2 (8 NeuronCores, data-parallel by chunk).

Reference computation (per full input):
    scores = (tanh(x @ W1 + b1) @ W2)[:, 0]          # [N]
    chunk_id = batch // 32                            # 32 graphs per chunk
    w = softmax of scores within each chunk           # [N]
    out = segment_sum(w[:, None] * x, batch)          # [4096, 256]

Shapes: x [262144, 256] f32, batch [262144] i64 (sorted, uniform: 64
nodes/graph), W1 [256,256], b1 [256], W2 [256,1].

Strategy (per core, 32768 nodes = 16 chunks of 2048 nodes):
  - host: ship x twice -- natural layout in bf16 (pooling; needs the
    precision) and transposed layout in fp8e4m3 (MLP scores; tolerates the
    quantization: the softmax weights absorb it, measured rel err ~1e-2
    against the 2e-2 gate).  24 MB/core instead of 32 MB bf16x2.
  - device, per chunk:
      hT = W1.T @ x8T         (PE, fp8 DoubleRow or bf16xfp8, psum f32)
      th = tanh(hT + b1)      (ACT, psum->sbuf bf16)
      s[n] = th.T @ W2        (PE, tanh tile as stationary operand -> s in
                               node-partition layout, F=1, nearly free)
      e = exp(s)              (ACT, one instr per chunk QUAD -- pairs and
                               singles for the tail chunks; DVE sums rows)
      D = allreduce(rowsum)   (GPSIMD partition_all_reduce, off critical path)
      E[n, g] = e * mask      (DVE, UNNORMALIZED; mask precomputed on host)
      outT[h,g] += x_t.T @ E  (PE, x_nat tile stationary, E moving F=32;
                               output transposed [256, 32] -- host undoes it)
      out = outT * (1/D)      (DVE, folded into the psum->sbuf output copy:
                               1/D is identical on every partition)
  - softmax max-subtraction is skipped: scores = tanh(.)@W2 are bounded by
    sum|W2| <= 16, so exp() cannot overflow in f32 and w = e/sum(e) is
    mathematically identical to the max-shifted form.
  - scheduling: the kernel is jointly DMA- and ACT-bound (both ~70us/core).
    The fp8 xt stream runs XT_LEAD chunks ahead of the bf16 x_nat stream on
    the serial DMA device so the tanh pipeline never starves; the last
    TAIL_SPLIT_CHUNKS x_nat slabs are quartered so the final poolings chase
    the arriving tiles, shortening the post-load serial tail; exp is
    quad-batched with pool-psum allocations staggered over two iterations
    to fit the 8-bank PSUM budget.
"""

import numpy as np
import ml_dtypes

import concourse.bass as bass
import concourse.bacc as bacc
import concourse.tile as tile
import concourse.mybir as mybir
import concourse.bass_isa as bass_isa
from concourse.bass_utils import run_bass_kernel_spmd

BF16 = mybir.dt.bfloat16
FP8 = mybir.dt.float8e4
F32 = mybir.dt.float32
NP_BF16 = ml_dtypes.bfloat16
NP_FP8 = ml_dtypes.float8_e4m3fn

N_CORES = 8
HIDDEN = 256
CHUNK_GRAPHS = 32
GRAPH_NODES = 64          # uniform: nodes per graph
TILE_NODES = 128          # nodes per node-tile (SBUF partition dim)
CHUNK_NODES = CHUNK_GRAPHS * GRAPH_NODES      # 2048
TILES_PER_CHUNK = CHUNK_NODES // TILE_NODES   # 16
BLOCK_NODES = 512
XT_LEAD = 6
TAIL_SPLIT_CHUNKS = 4
TAIL_SPLIT_WAYS = 4

# MLP matmul flavor: "mixed" = bf16 W1 stationary x fp8 moving;
# "dr" = fp8 DoubleRow (both operands fp8, K=256 in one matmul, 2x rate)
MLP_MODE = "dr"

_NC_CACHE = {}


def build_nc(n_chunks, mlp_mode=MLP_MODE):
    """Build the per-core Bass program (identical across cores)."""
    assert n_chunks % 2 == 0
    nc = bacc.Bacc("TRN2", target_bir_lowering=False, debug=False,
                   enable_asserts=False)

    nodes = n_chunks * CHUNK_NODES
    # DRAM I/O (per-core shard)
    x_nat_d = nc.dram_tensor(
        "x_nat", [n_chunks, TILE_NODES, TILES_PER_CHUNK, HIDDEN], BF16,
        kind="ExternalInput").ap()
    # transposed fp8 layout: [q, kt, node] with k = kt*128 + q
    x8t_d = nc.dram_tensor(
        "x8t", [128, 2, nodes], FP8, kind="ExternalInput").ap()
    w1_dt = FP8 if mlp_mode == "dr" else BF16
    w1_d = nc.dram_tensor("w1", [128, 2, 2, 128], w1_dt,
                          kind="ExternalInput").ap()
    w2_d = nc.dram_tensor("w2", [128, 2], BF16, kind="ExternalInput").ap()
    b1_d = nc.dram_tensor("b1", [128, 2], F32, kind="ExternalInput").ap()
    mask_d = nc.dram_tensor(
        "maskw", [TILE_NODES, CHUNK_GRAPHS, TILES_PER_CHUNK], BF16,
        kind="ExternalInput").ap()
    # transposed output, two chunks per row-group: [cpair, p=h, ci, kt, g]
    out_d = nc.dram_tensor(
        "outt", [n_chunks // 2, 128, 2, 2, CHUNK_GRAPHS], F32,
        kind="ExternalOutput").ap()

    with tile.TileContext(nc) as tc:
        with (
            tc.tile_pool(name="consts", bufs=1) as consts,
            tc.tile_pool(name="xpool", bufs=7) as xpool,
            tc.tile_pool(name="xtpool", bufs=9) as xtpool,
            tc.tile_pool(name="thpool", bufs=10) as thpool,
            tc.tile_pool(name="epool", bufs=5) as epool,
            tc.tile_pool(name="opool", bufs=8) as opool,
            tc.tile_pool(name="hpsum", bufs=2, space="PSUM") as hpsum,
            tc.tile_pool(name="spsum", bufs=2, space="PSUM") as spsum,
            tc.tile_pool(name="ppsum", bufs=2, space="PSUM") as ppsum,
        ):
            st = {}        # per-chunk live tiles
            spair = {}     # per-pair score psum tiles

            HALF = CHUNK_NODES // 2

            def load_xt(c):
                # one full-slab DMA per chunk: a 1.46us transfer outruns the
                # ~1.2us single-queue dispatch cadence, so the serial DMA
                # device never starves during the xt-lead fill (half-slab
                # transfers did starve it).  The MLP still consumes by
                # half-slices of the one tile.
                t = xtpool.tile([128, 2, CHUNK_NODES], FP8, tag="xt",
                                name=f"xt{c}")
                lo = c * CHUNK_NODES
                nc.sync.dma_start(out=t, in_=x8t_d[:, :, lo:lo + CHUNK_NODES])
                st[c] = {"xt": (t[:, :, 0:HALF], t[:, :, HALF:]), "th": {}}

            def load_xnat(c):
                x_sb = xpool.tile([TILE_NODES, TILES_PER_CHUNK, HIDDEN], BF16,
                                  tag="x")
                if c >= n_chunks - TAIL_SPLIT_CHUNKS:
                    # tail chunks: sliced loads so their pooling matmuls
                    # chase the arriving tiles instead of waiting the slab
                    q = TILES_PER_CHUNK // TAIL_SPLIT_WAYS
                    for qi in range(TAIL_SPLIT_WAYS):
                        nc.sync.dma_start(
                            out=x_sb[:, qi * q:(qi + 1) * q],
                            in_=x_nat_d[c, :, qi * q:(qi + 1) * q])
                else:
                    nc.sync.dma_start(out=x_sb, in_=x_nat_d[c])
                st[c]["x"] = x_sb

            # first MLP inputs go ahead of everything else on the SP queue;
            # consts dispatch concurrently through the Activation HWDGE queue
            # chunk 0: w1 rides the serial DMA stream between the two xt
            # halves, so the first MLP matmul (xt half 0 + w1) starts ~1.3us
            # earlier than if w1 queued behind both halves
            xt0 = xtpool.tile([128, 2, CHUNK_NODES], FP8, tag="xt",
                              name="xt0")
            nc.sync.dma_start(out=xt0, in_=x8t_d[:, :, 0:CHUNK_NODES])
            w1_sb = consts.tile([128, 2, 2, 128], w1_dt)
            nc.sync.dma_start(out=w1_sb, in_=w1_d)
            st[0] = {"xt": (xt0[:, :, 0:HALF], xt0[:, :, HALF:]), "th": {}}
            b1_sb = consts.tile([128, 2], F32)
            nc.scalar.dma_start(out=b1_sb, in_=b1_d)
            # activation-table preload: dummy tanh+exp on an already-loaded
            # const so the (shared) table load hides under the DMA fill
            scratch = consts.tile([128, 1], BF16, name="scratch")
            nc.scalar.activation(scratch, b1_sb[:, 0:1],
                                 mybir.ActivationFunctionType.Tanh)
            nc.scalar.activation(scratch, b1_sb[:, 0:1],
                                 mybir.ActivationFunctionType.Exp)
            # w2 (first used by scores in iteration 1) and the mask (first
            # used by the iteration-2 softmax) are dispatched from inside
            # iteration 1: their tiny transfers would otherwise steal HWDGE
            # dispatch slots from the xt stream in the startup window
            w2_sb = consts.tile([128, 2], BF16, name="w2_sb")
            mask_sb = consts.tile([TILE_NODES, CHUNK_GRAPHS, TILES_PER_CHUNK],
                                  BF16)
            # xt runs XT_LEAD chunks ahead of xnat on the wire: the tanh
            # stream (the critical engine) consumes xt at almost exactly the
            # DMA rate, so the lead absorbs the accumulated per-chunk DMA
            # deficit (stores, dispatch bubbles) that would otherwise stall
            # ACT in the back half.  xnat still flows every iteration so the
            # pooling matmuls never clog the PE queue waiting for it.
            if n_chunks > 1:
                load_xt(1)
            load_xnat(0)
            for cc in range(2, min(XT_LEAD + 1, n_chunks)):
                load_xt(cc)
            if n_chunks > 1:
                load_xnat(1)

            # Software pipeline (pair-structured):
            #   iteration c emits, round-robin per node-tile slot:
            #     MLP matmuls of chunk c        (+ tanh on ACT)
            #     score matmuls of c-1          (F=1, nearly free)
            #     pooling matmuls of pair (c-3, c-2) on odd c
            #   after the slots: softmax of pair (c-2, c-1) on even c,
            #   output store of pair (c-3, c-2) on odd c.

            # chunks below QEND batch their softmax per QUAD (one exp for
            # 4 chunks): fewer ACT instructions on the critical tanh stream
            QEND = ((n_chunks - 4) // 4) * 4 if n_chunks >= 8 else 0

            def mlp_gen(c):
                """Yields after each MLP matmul slot (16 total)."""
                if c < QEND:
                    if c % 4 == 0:
                        spair[("q", c // 4)] = spsum.tile(
                            [128, 4, TILES_PER_CHUNK], F32, tag="s",
                            name=f"s_q{c // 4}")
                elif c % 2 == 0:
                    spair[c // 2] = spsum.tile([128, 2, TILES_PER_CHUNK], F32,
                                               tag="s", name=f"s_ps{c // 2}")
                for bp in range(2):          # block pair: nodes [bp*1024, ...)
                    xt_sb = st[c]["xt"][bp]
                    for mt in range(2):
                        h_ps = hpsum.tile([128, 2, BLOCK_NODES], F32, tag="h",
                                          name=f"h_ps{c}_{bp}_{mt}")
                        for bb in range(2):
                            nlo = bb * BLOCK_NODES
                            if mlp_mode == "dr":
                                nc.tensor.matmul(
                                    h_ps[:, bb, :], w1_sb[:, :, mt, :],
                                    xt_sb[:, :, nlo:nlo + BLOCK_NODES],
                                    perf_mode=mybir.MatmulPerfMode.DoubleRow,
                                    start=True, stop=True)
                                yield
                                yield
                            else:
                                for kt in range(2):
                                    nc.tensor.matmul(
                                        h_ps[:, bb, :], w1_sb[:, kt, mt, :],
                                        xt_sb[:, kt, nlo:nlo + BLOCK_NODES],
                                        start=(kt == 0), stop=(kt == 1))
                                    yield
                        th = thpool.tile([128, 2, BLOCK_NODES], BF16, tag="th",
                                         name=f"th{c}_{bp}_{mt}")
                        nc.scalar.activation(
                            th, h_ps, mybir.ActivationFunctionType.Tanh,
                            bias=b1_sb[:, mt:mt + 1], scale=1.0)
                        st[c]["th"][(bp, mt)] = th

            def score_ops(c):
                """16 slots; each slot t emits 2 accumulating F=1 matmuls
                (tanh tile as stationary operand)."""
                if c < QEND:
                    s_ps = spair[("q", c // 4)][:, c % 4]
                else:
                    s_ps = spair[c // 2][:, c % 2]
                ops = []
                for t in range(TILES_PER_CHUNK):
                    b, tl = divmod(t, 4)
                    bp, bb = divmod(b, 2)

                    def op(mt, t=t, bp=bp, bb=bb, c=c, tl=tl):
                        th = st[c]["th"][(bp, mt)]
                        nc.tensor.matmul(
                            s_ps[:, t:t + 1],
                            th[:, bb, tl * 128:(tl + 1) * 128],
                            w2_sb[:, mt:mt + 1],
                            start=(mt == 0), stop=(mt == 1))
                    ops.append((lambda op=op: op(0), lambda op=op: op(1)))
                return ops

            def _softmax_tail(c, e_view, rden, ci, alloc_p=True):
                # E' = e * mask, UNNORMALIZED: 1/D is folded into the
                # psum->sbuf output copy instead (rden holds the same value
                # on every partition), so pooling never waits the
                # allreduce/reciprocal chain
                e_full = epool.tile(
                    [TILE_NODES, CHUNK_GRAPHS, TILES_PER_CHUNK], BF16,
                    tag="efull", name=f"efull{c}")
                e_bc = e_view.unsqueeze(1).broadcast_to(
                    [TILE_NODES, CHUNK_GRAPHS, TILES_PER_CHUNK])
                nc.vector.tensor_mul(e_full, e_bc, mask_sb)
                st[c]["E"] = e_full
                st[c]["rden"] = (rden, ci)
                if alloc_p:
                    emit_palloc(c)

            def emit_palloc(c):
                p_ps = ppsum.tile([128, 2, CHUNK_GRAPHS], F32, tag="p",
                                  name=f"p_ps{c}")
                st[c]["p"] = p_ps

            def emit_softmax_quad(qi):
                """exp + normalization for chunks 4qi..4qi+3 in one batch.
                Only the first two chunks get their pool-psum tiles now; the
                other two are allocated next iteration (8-bank budget)."""
                e_sb = epool.tile([128, 4, TILES_PER_CHUNK], BF16, tag="e",
                                  name=f"e_q{qi}")
                nc.scalar.activation(
                    e_sb, spair[("q", qi)], mybir.ActivationFunctionType.Exp)
                del spair[("q", qi)]
                acc = epool.tile([128, 4], F32, tag="acc", name=f"acc_q{qi}")
                nc.vector.tensor_reduce(
                    acc, e_sb, mybir.AxisListType.X, mybir.AluOpType.add)
                dsum = epool.tile([128, 4], F32, tag="dsum",
                                  name=f"dsum_q{qi}")
                nc.gpsimd.partition_all_reduce(
                    dsum, acc, 128, bass_isa.ReduceOp.add)
                rden = epool.tile([128, 4], F32, tag="rden",
                                  name=f"rden_q{qi}")
                nc.vector.reciprocal(rden, dsum)
                for ci in range(4):
                    _softmax_tail(4 * qi + ci, e_sb[:, ci], rden, ci,
                                  alloc_p=(ci < 2))

            def emit_softmax_pair(p):
                """exp + normalization for chunks (2p, 2p+1) in one batch."""
                e_sb = epool.tile([128, 2, TILES_PER_CHUNK], BF16, tag="e",
                                  name=f"e_sb{p}")
                nc.scalar.activation(
                    e_sb, spair[p], mybir.ActivationFunctionType.Exp)
                del spair[p]
                acc = epool.tile([128, 2], F32, tag="acc")
                nc.vector.tensor_reduce(
                    acc, e_sb, mybir.AxisListType.X, mybir.AluOpType.add)
                dsum = epool.tile([128, 2], F32, tag="dsum")
                nc.gpsimd.partition_all_reduce(
                    dsum, acc, 128, bass_isa.ReduceOp.add)
                rden = epool.tile([128, 2], F32, tag="rden")
                nc.vector.reciprocal(rden, dsum)
                for ci in range(2):
                    _softmax_tail(2 * p + ci, e_sb[:, ci], rden, ci)

            def emit_softmax_single(c):
                """Per-chunk softmax for the tail chunks: avoids coupling the
                last chunks' pooling to the final pair exp."""
                p, ci = divmod(c, 2)
                e_sb = epool.tile([128, TILES_PER_CHUNK], BF16, tag="e",
                                  name=f"e_sb_s{c}")
                nc.scalar.activation(
                    e_sb, spair[p][:, ci], mybir.ActivationFunctionType.Exp)
                if ci == 1:
                    del spair[p]
                acc = epool.tile([128, 1], F32, tag="acc", name=f"acc_s{c}")
                nc.vector.tensor_reduce(
                    acc, e_sb, mybir.AxisListType.X, mybir.AluOpType.add)
                dsum = epool.tile([128, 1], F32, tag="dsum", name=f"dsum_s{c}")
                nc.gpsimd.partition_all_reduce(
                    dsum, acc, 128, bass_isa.ReduceOp.add)
                rden = epool.tile([128, 1], F32, tag="rden", name=f"rden_s{c}")
                nc.vector.reciprocal(rden, dsum)
                _softmax_tail(c, e_sb, rden, 0)

            def pool_chunk_ops(cs):
                """16 slots; slot t emits 2 matmuls per chunk in cs (kt
                halves): stationary = x_nat tile [128 nodes, 128 h], moving =
                E[:, :, t] (F=32), accumulated over the 16 node-tiles into
                outT [128, 2, 32] per chunk.  Both kt slices share one 2KB
                psum zero region, so exactly one start and one stop per
                chunk tile: a second start would re-mark the region
                pending-zero and wipe the first group's partial sums."""
                ops = []
                for t in range(TILES_PER_CHUNK):
                    def op(t=t):
                        for c in cs:
                            for kt in range(2):
                                nc.tensor.matmul(
                                    st[c]["p"][:, kt, :],
                                    st[c]["x"][:, t, kt * 128:(kt + 1) * 128],
                                    st[c]["E"][:, :, t],
                                    start=(t == 0 and kt == 0),
                                    stop=(t == TILES_PER_CHUNK - 1 and
                                          kt == 1),
                                    skip_group_check=True)
                    ops.append(op)
                return ops

            deferred = []

            def emit_store_pair(p):
                o2 = opool.tile([128, 2, 2, CHUNK_GRAPHS], F32,
                                tag="o", name=f"o2_{p}")
                for ci in range(2):
                    rden, ri = st[2 * p + ci]["rden"]
                    nc.vector.tensor_scalar_mul(
                        o2[:, ci], st[2 * p + ci]["p"], rden[:, ri:ri + 1])
                # defer ALL pair-store dispatches until the last pair's
                # copies are emitted: early stores would steal serial
                # DMA-engine slots from the load stream and delay the last
                # load byte, which gates the terminal pool chain.  By then
                # the SP queue has no loads left to block.
                deferred.append((p, o2))
                if p == lastp - 1:
                    for dp, do2 in deferred:
                        nc.sync.dma_start(out=out_d[dp], in_=do2)
                    deferred.clear()
                del st[2 * p]
                del st[2 * p + 1]

            lastp = n_chunks // 2 - 1
            for c in range(n_chunks + 2):
                if c == min(1, n_chunks - 1):
                    nc.scalar.dma_start(out=w2_sb, in_=w2_d)
                    nc.scalar.dma_start(out=mask_sb, in_=mask_d)
                if c + XT_LEAD + 1 < n_chunks:
                    load_xt(c + XT_LEAD + 1)
                if c + 2 < n_chunks:
                    load_xnat(c + 2)
                # Sequential emission per iteration: PE executes in order, so
                # score ops (which wait on tanh of c-1) must not sit ahead of
                # MLP matmuls whose inputs are already resident.
                if c < n_chunks:
                    for _ in mlp_gen(c):
                        pass
                if 1 <= c <= n_chunks:
                    for op0, op1 in score_ops(c - 1):
                        op0()
                        op1()
                if c % 4 == 1 and 5 <= c <= QEND + 1:
                    # quad first half: chunks (c-5, c-4)
                    for op in pool_chunk_ops([c - 5, c - 4]):
                        op()
                    emit_store_pair((c - 5) // 2)
                    emit_palloc(c - 3)
                    emit_palloc(c - 2)
                elif c % 4 == 2 and 6 <= c <= QEND + 2:
                    # quad second half: chunks (c-4, c-3)
                    for op in pool_chunk_ops([c - 4, c - 3]):
                        op()
                    emit_store_pair((c - 4) // 2)
                elif c >= 3 and c % 2 == 1 and QEND // 2 <= (c - 3) // 2 < lastp:
                    for op in pool_chunk_ops([c - 3, c - 2]):
                        op()
                    emit_store_pair((c - 3) // 2)
                elif c == n_chunks:
                    for op in pool_chunk_ops([n_chunks - 2]):
                        op()
                elif c == n_chunks + 1:
                    for op in pool_chunk_ops([n_chunks - 1]):
                        op()
                if c % 4 == 0 and 4 <= c <= QEND:
                    emit_softmax_quad((c - 4) // 4)
                if c >= 2 and c % 2 == 0 and QEND // 2 <= (c - 2) // 2 < lastp:
                    emit_softmax_pair((c - 2) // 2)
                if c == n_chunks - 1:
                    emit_softmax_single(n_chunks - 2)
                if c == n_chunks:
                    # copy chunk n-2's pooled output while n-1 still cooks
                    o2l = opool.tile([128, 2, 2, CHUNK_GRAPHS], F32,
                                     tag="o", name="o2_last")
                    st["o2l"] = o2l
                    rden, ri = st[n_chunks - 2]["rden"]
                    nc.vector.tensor_scalar_mul(
                        o2l[:, 0], st[n_chunks - 2]["p"], rden[:, ri:ri + 1])
                    emit_softmax_single(n_chunks - 1)
                if c == n_chunks + 1:
                    o2l = st["o2l"]
                    rden, ri = st[n_chunks - 1]["rden"]
                    nc.vector.tensor_scalar_mul(
                        o2l[:, 1], st[n_chunks - 1]["p"], rden[:, ri:ri + 1])
                    # final store via SP HWDGE: lower latency than SWDGE and
                    # nothing is left in the SP queue to block
                    nc.sync.dma_start(out=out_d[lastp], in_=o2l)

    nc.compile()
    return nc


def _prep_inputs(x, W1, b1, W2, n_chunks_per_core, mlp_mode=MLP_MODE):
    """Host-side marshalling: casts, layouts, masks. Returns in_maps."""
    N, H = x.shape
    nodes_per_core = n_chunks_per_core * CHUNK_NODES

    xf = np.asarray(x, dtype=np.float32)
    xb = xf.astype(NP_BF16)

    # natural layout: [core, chunk, p, t, h]  (bf16: pooling precision)
    x_nat = np.ascontiguousarray(
        xb.reshape(N_CORES, n_chunks_per_core, TILES_PER_CHUNK, TILE_NODES, H)
        .transpose(0, 1, 3, 2, 4))
    # transposed fp8 layout: [core, q, kt, n_local] with k = kt*128 + q
    x8t = np.ascontiguousarray(
        xf.reshape(N_CORES, nodes_per_core, 2, 128)
        .transpose(0, 3, 2, 1)).astype(NP_FP8)

    np_w1 = NP_FP8 if mlp_mode == "dr" else NP_BF16
    W1c = np.asarray(W1, dtype=np.float32).astype(np_w1)     # [hin, hout]
    w1_host = np.ascontiguousarray(
        W1c.reshape(2, 128, 2, 128).transpose(1, 0, 2, 3))  # [q, kt, mt, j]
    w2_host = np.ascontiguousarray(
        np.asarray(W2).astype(NP_BF16).reshape(2, 128).T)   # [p, mt]
    b1_host = np.ascontiguousarray(
        np.asarray(b1).astype(np.float32).reshape(2, 128).T)  # [p, mt]

    # mask[p, g, t] = 1 iff node (t, p) of a chunk belongs to graph g
    p_idx = np.arange(TILE_NODES)
    t_idx = np.arange(TILES_PER_CHUNK)
    g_of_pt = 2 * t_idx[None, :] + p_idx[:, None] // GRAPH_NODES  # [p, t]
    mask_host = (g_of_pt[:, None, :] ==
                 np.arange(CHUNK_GRAPHS)[None, :, None]).astype(NP_BF16)

    in_maps = []
    for core in range(N_CORES):
        in_maps.append({
            "x_nat": x_nat[core],
            "x8t": x8t[core],
            "w1": w1_host,
            "w2": w2_host,
            "b1": b1_host,
            "maskw": mask_host,
        })
    return in_maps


def _reference_numpy(x, batch, W1, b1, W2):
    """Fallback for non-uniform batch layouts: straight numpy."""
    x = np.asarray(x, dtype=np.float64)
    batch = np.asarray(batch).astype(np.int64)
    # the reference uses a fixed segment count (num_graphs = num_nodes/64),
    # not batch.max()+1 -- keep trailing empty graphs as zero rows
    n_graphs = max(int(batch.max()) + 1, x.shape[0] // GRAPH_NODES)
    scores = np.tanh(x @ np.asarray(W1, np.float64) +
                     np.asarray(b1, np.float64)) @ np.asarray(W2, np.float64)
    scores = scores[:, 0]
    chunk_id = batch // CHUNK_GRAPHS
    n_chunks = int(chunk_id.max()) + 1
    m = np.full(n_chunks, -np.inf)
    np.maximum.at(m, chunk_id, scores)
    e = np.exp(scores - m[chunk_id])
    denom = np.zeros(n_chunks)
    np.add.at(denom, chunk_id, e)
    w = e / denom[chunk_id]
    out = np.zeros((n_graphs, x.shape[1]))
    np.add.at(out, batch, w[:, None] * x)
    return out.astype(np.float32)


def kernel(x, batch, W1, b1, W2, trace=False):
    x = np.asarray(x)
    batch = np.asarray(batch)
    N, H = x.shape
    n_graphs = int(batch[-1]) + 1

    # This kernel is specialized for the uniform sorted batch that the
    # reference generator produces (64 nodes per graph). Anything else
    # falls back to a host computation.
    expected = (np.arange(N, dtype=np.int64) * n_graphs) // N
    if (H != HIDDEN or N % (N_CORES * CHUNK_NODES) != 0
            or n_graphs % (N_CORES * CHUNK_GRAPHS) != 0
            or (N // (N_CORES * CHUNK_NODES)) % 2 != 0
            or not np.array_equal(batch.astype(np.int64), expected)):
        return _reference_numpy(x, batch, W1, b1, W2)

    n_chunks_per_core = N // (N_CORES * CHUNK_NODES)

    key = (n_chunks_per_core, MLP_MODE)
    if key not in _NC_CACHE:
        _NC_CACHE[key] = build_nc(n_chunks_per_core, mlp_mode=MLP_MODE)
    nc = _NC_CACHE[key]

    in_maps = _prep_inputs(x, W1, b1, W2, n_chunks_per_core,
                           mlp_mode=MLP_MODE)
    try:
        res = run_bass_kernel_spmd(nc, in_maps, core_ids=list(range(N_CORES)),
                                   trace=trace)
    except ModuleNotFoundError:
        # NTFF trace hooks unavailable in this environment
        res = run_bass_kernel_spmd(nc, in_maps, core_ids=list(range(N_CORES)),
                                   trace=False)
    # outt [cpair, p, ci, kt, g] -> out[g, h]: g=(2cp+ci)*32+gg, h=kt*128+p
    outs = []
    for r in res.results:
        arr = r["outt"]
        outs.append(np.ascontiguousarray(
            arr.transpose(0, 2, 4, 3, 1)).reshape(-1, HIDDEN))
    out = np.concatenate(outs, axis=0)
    if trace:
        kernel.last_results = res
    return out.astype(np.float32)

